# revision 17
# baseline (speedup 1.0000x reference)
"""Trainium2 Bass kernel for nn_Encoder_88983132439258 (GNN message passing).

Strategy (8 NeuronCores, data-parallel over destination nodes = graphs):
  - Each core owns 8192 destination nodes (= 2 complete graphs).
  - Host pre-builds a padded slot table (dst-major CSR padded to PAD slots per
    dst) so the per-layer edge aggregation becomes:
        indirect-DMA gather of z[src] rows  ->  DVE w-scale  ->  DVE slot-reduce
  - Weights are folded into the gather table: the table for layer l holds
    z_l = h_l @ W_l, so a conv layer is  h_{l+1} = ELU(segsum(w * z_l[src]) + b_l).
  - Per-layer epilogue runs feature-major via PE transposes and a block-diagonal
    (4x32x32) weight matmul, then the new table is AllGathered across cores.
  - FC head: h4 is AllGathered as H [16, 131072]; each core computes a 32-wide
    column shard of FC1 and its FC2 partial; partials are AllReduced.
"""

import numpy as np
import ml_dtypes

import concourse.bacc as bacc
import concourse.mybir as mybir
import concourse.tile as tile
import concourse.bass as bass
from concourse import bass_utils

F32 = mybir.dt.float32
BF16 = mybir.dt.bfloat16
I32 = mybir.dt.int32

N = 65536
NODES_PER = 4096
N_GRAPHS = 16
E_TOT = 2097152
FEAT_IN = 16
HID = 32
FC_HID = 256
LATENT = 64
NC = 8                 # cores
OWN = N // NC          # 8192 dsts per core
NCHUNK = 16            # dst chunks per core
CHD = OWN // NCHUNK    # 512 dsts per chunk
NB = CHD // 128        # 4 dst blocks of 128 per chunk
P = 128

_prog_cache = {}


def _build_program(PAD):
    """One SPMD program for all 8 cores; per-core data comes via inputs."""
    PAD4 = PAD * NB
    nc = bacc.Bacc("TRN2", target_bir_lowering=False, debug=False,
                   num_devices=NC)

    # ---- I/O ----
    tab1 = nc.dram_tensor("tab1", [N + 1, HID], BF16, kind="ExternalInput")
    idx_in = nc.dram_tensor("idx", [NCHUNK, P, PAD4], I32,
                            kind="ExternalInput")
    w_in = nc.dram_tensor("warr", [NCHUNK, P, PAD4], BF16,
                          kind="ExternalInput")
    wbd_in = nc.dram_tensor("wbd", [2, P, P], BF16, kind="ExternalInput")
    bst_in = nc.dram_tensor("bst", [3, P], F32, kind="ExternalInput")
    idf_in = nc.dram_tensor("identf", [P, P], F32, kind="ExternalInput")
    idb_in = nc.dram_tensor("identb", [P, P], BF16, kind="ExternalInput")
    wfc1_in = nc.dram_tensor("wfc1s", [P, 1024 * HID], BF16,
                             kind="ExternalInput")
    wfc2_in = nc.dram_tensor("wfc2s", [HID, LATENT], F32,
                             kind="ExternalInput")
    bfc1_in = nc.dram_tensor("bfc1t", [N_GRAPHS, HID], F32,
                             kind="ExternalInput")
    bfc2_in = nc.dram_tensor("bfc2t", [N_GRAPHS, LATENT], F32,
                             kind="ExternalInput")
    out = nc.dram_tensor("out", [N_GRAPHS, LATENT], F32,
                         kind="ExternalOutput")

    # ---- internal DRAM ----
    tab2 = nc.dram_tensor("tab2", [N + 1, HID], BF16, addr_space="Shared")
    tab3 = nc.dram_tensor("tab3", [N + 1, HID], BF16, addr_space="Shared")
    cin2 = nc.dram_tensor("cin2", [OWN, HID], BF16)
    cin3 = nc.dram_tensor("cin3", [OWN, HID], BF16)
    hin = nc.dram_tensor("hin", [2, NODES_PER * HID], BF16)
    hfull = nc.dram_tensor("hfull", [N_GRAPHS, NODES_PER * HID], BF16, addr_space="Shared")
    arin = nc.dram_tensor("arin", [N_GRAPHS, LATENT], F32)
    arout = nc.dram_tensor("arout", [N_GRAPHS, LATENT], F32, addr_space="Shared")

    groups = [list(range(NC))]

    with tile.TileContext(nc) as tc:
        with tc.tile_pool(name="const", bufs=1) as cst, \
             tc.tile_pool(name="work", bufs=2) as wk, \
             tc.tile_pool(name="small", bufs=3) as sm, \
             tc.tile_pool(name="ps1", bufs=2, space="PSUM") as ps1, \
             tc.tile_pool(name="ps2", bufs=1, space="PSUM") as ps2, \
             tc.tile_pool(name="psfc", bufs=1, space="PSUM") as psfc, \
             tc.tile_pool(name="fcp", bufs=4) as fcp:

            # ---- constants to SBUF ----
            wbd_t = [cst.tile([P, P], BF16, tag=f"wbd{i}", name=f"wbd_t{i}")
                     for i in range(2)]
            for i in range(2):
                nc.sync.dma_start(out=wbd_t[i][:], in_=wbd_in[i])
            bst_t = [cst.tile([P, 1], F32, tag=f"bst{i}", name=f"bst_t{i}")
                     for i in range(3)]
            for i in range(3):
                nc.sync.dma_start(out=bst_t[i][:],
                                  in_=bst_in[i].rearrange("(p o) -> p o", o=1))
            idf_t = cst.tile([P, P], F32, tag="idf")
            nc.sync.dma_start(out=idf_t[:], in_=idf_in[:, :])
            idb_t = cst.tile([P, P], BF16, tag="idb")
            nc.sync.dma_start(out=idb_t[:], in_=idb_in[:, :])
            wfc2_t = cst.tile([HID, LATENT], F32, tag="wfc2")
            nc.sync.dma_start(out=wfc2_t[:], in_=wfc2_in[:, :])
            bfc1_t = cst.tile([N_GRAPHS, HID], F32, tag="bfc1")
            nc.sync.dma_start(out=bfc1_t[:], in_=bfc1_in[:, :])
            bfc2_t = cst.tile([N_GRAPHS, LATENT], F32, tag="bfc2")
            nc.sync.dma_start(out=bfc2_t[:], in_=bfc2_in[:, :])

            # zero row at index N for padding slots of tab2/tab3
            zrow = cst.tile([1, HID], BF16, tag="zrow")
            nc.gpsimd.memset(zrow[:], 0.0)
            nc.sync.dma_start(out=tab2[N:N + 1, :], in_=zrow[:])
            nc.sync.dma_start(out=tab3[N:N + 1, :], in_=zrow[:])

            # ---- conv layers ----
            def layer(li, table_ap, next_store):
                """li: 0,1,2. next_store(chunk_c, tnode_sbuf_tile)."""
                for c in range(NCHUNK):
                    it = sm.tile([P, PAD4], I32, tag="it")
                    nc.sync.dma_start(out=it[:], in_=idx_in[c])
                    wt = sm.tile([P, PAD4], BF16, tag="wt")
                    nc.sync.dma_start(out=wt[:], in_=w_in[c])
                    g = wk.tile([P, PAD4 * HID], BF16, tag="g")
                    for sl in range(PAD4):
                        nc.gpsimd.indirect_dma_start(
                            out=g[:, sl * HID:(sl + 1) * HID],
                            out_offset=None,
                            in_=table_ap,
                            in_offset=bass.IndirectOffsetOnAxis(
                                ap=it[:, sl:sl + 1], axis=0),
                        )
                    m = wk.tile([P, PAD4 * HID], BF16, tag="m")
                    nc.vector.tensor_tensor(
                        out=m[:].rearrange("p (c f) -> p c f", f=HID),
                        in0=g[:].rearrange("p (c f) -> p c f", f=HID),
                        in1=wt[:].to_broadcast(
                            [P, PAD4, HID]),
                        op=mybir.AluOpType.mult,
                    )
                    # tree-fold over s (outermost free factor), contiguous
                    BF = NB * HID
                    half = PAD // 2
                    m2 = wk.tile([P, half * BF], F32, tag="m2")
                    nc.vector.tensor_add(out=m2[:], in0=m[:, :half * BF],
                                         in1=m[:, half * BF:])
                    cur = half
                    while cur > 1:
                        h2 = cur // 2
                        nc.vector.tensor_add(out=m2[:, :h2 * BF],
                                             in0=m2[:, :h2 * BF],
                                             in1=m2[:, h2 * BF:cur * BF])
                        cur = h2
                    # ---- epilogue: [128 dst, (b f)] -> feature-major ----
                    pt = ps1.tile([P, P], F32, tag="pt")
                    nc.tensor.transpose(out=pt[:], in_=m2[:, :BF],
                                        identity=idf_t[:])
                    # ELU(v) with v = pt + b:  relu(v) + exp(min(v,0)) - 1
                    rl = sm.tile([P, P], BF16, tag="rl")
                    nc.scalar.activation(rl[:], pt[:],
                                         mybir.ActivationFunctionType.Relu,
                                         bias=bst_t[li][:])
                    mn = sm.tile([P, P], F32, tag="mn")
                    nc.vector.scalar_tensor_tensor(
                        out=mn[:], in0=pt[:], scalar=bst_t[li][:],
                        in1=rl[:], op0=mybir.AluOpType.add,
                        op1=mybir.AluOpType.subtract)
                    ex = sm.tile([P, P], F32, tag="ex")
                    nc.scalar.activation(ex[:], mn[:],
                                         mybir.ActivationFunctionType.Exp)
                    if li < 2:
                        h = sm.tile([P, P], BF16, tag="h")
                    else:
                        h = sm.tile([P, P], F32, tag="hf")
                    nc.vector.scalar_tensor_tensor(
                        out=h[:], in0=rl[:], scalar=-1.0, in1=ex[:],
                        op0=mybir.AluOpType.add, op1=mybir.AluOpType.add)
                    if li < 2:
                        # z = h @ W_{l+1} via block-diag W, still feat-major
                        pz = ps2.tile([P, P], F32, tag="pz")
                        nc.tensor.matmul(out=pz[:], lhsT=wbd_t[li][:],
                                         rhs=h[:], start=True, stop=True)
                        zs = sm.tile([P, P], F32, tag="zs")
                        nc.scalar.copy(zs[:], pz[:])
                        pn = ps1.tile([P, P], F32, tag="pn")
                        nc.tensor.transpose(out=pn[:], in_=zs[:],
                                            identity=idf_t[:])
                        tn = sm.tile([P, P], BF16, tag="tn")
                        nc.vector.tensor_copy(tn[:], pn[:])
                    else:
                        pn = ps1.tile([P, P], F32, tag="pn")
                        nc.tensor.transpose(out=pn[:], in_=h[:],
                                            identity=idf_t[:])
                        tn = sm.tile([P, P], BF16, tag="tn")
                        nc.vector.tensor_copy(tn[:], pn[:])
                    next_store(c, tn)

            # layer 1
            def store_l1(c, tn):
                nc.sync.dma_start(
                    out=cin2.ap()[c * CHD:(c + 1) * CHD, :].rearrange(
                        "(b p) f -> p b f", p=P),
                    in_=tn[:].rearrange("p (b f) -> p b f", f=HID))
            layer(0, tab1.ap(), store_l1)
            nc.gpsimd.collective_compute(
                "AllGather", mybir.AluOpType.bypass, replica_groups=groups,
                ins=[cin2.ap().opt()], outs=[tab2.ap()[:N, :].opt()])

            # layer 2
            def store_l2(c, tn):
                nc.sync.dma_start(
                    out=cin3.ap()[c * CHD:(c + 1) * CHD, :].rearrange(
                        "(b p) f -> p b f", p=P),
                    in_=tn[:].rearrange("p (b f) -> p b f", f=HID))
            layer(1, tab2.ap(), store_l2)
            nc.gpsimd.collective_compute(
                "AllGather", mybir.AluOpType.bypass, replica_groups=groups,
                ins=[cin3.ap().opt()], outs=[tab3.ap()[:N, :].opt()])

            # layer 3 -> H rows (2 graphs per core)
            def store_l3(c, tn):
                # tn[p, (b f)] -> h4[dloc = c*CHD + b*128 + p, f]
                nc.sync.dma_start(
                    out=hin.ap().rearrange("g (i f) -> (g i) f", f=HID)[
                        c * CHD:(c + 1) * CHD, :]
                    .rearrange("(b p) f -> p b f", p=P),
                    in_=tn[:].rearrange("p (b f) -> p b f", f=HID))
            layer(2, tab3.ap(), store_l3)
            nc.gpsimd.collective_compute(
                "AllGather", mybir.AluOpType.bypass, replica_groups=groups,
                ins=[hin.ap().opt()], outs=[hfull.ap().opt()])

            # ---- FC head ----
            # FC1: accumulate over 1024 dim-chunks of 128
            pfc = psfc.tile([N_GRAPHS, HID], F32, tag="pfc")
            NGRP = 8       # wfc1 chunks loaded per DMA
            NSLAB = 64     # H chunks per slab
            for cg in range(1024 // NGRP):
                wc = fcp.tile([P, NGRP * HID], BF16, tag="wc")
                nc.sync.dma_start(
                    out=wc[:],
                    in_=wfc1_in.ap()[:, cg * NGRP * HID:(cg + 1) * NGRP * HID])
                for j in range(NGRP):
                    ci = cg * NGRP + j
                    if ci % NSLAB == 0:
                        slab = fcp.tile([N_GRAPHS, NSLAB * P], BF16,
                                        tag="slab", name=f"slab{ci}")
                        nc.sync.dma_start(
                            out=slab[:],
                            in_=hfull.ap()[:, ci * P:(ci + NSLAB) * P])
                    pt16 = ps2.tile([P, N_GRAPHS], BF16, tag="pz",
                                    name=f"pt16_{ci}")
                    nc.tensor.transpose(
                        out=pt16[:],
                        in_=slab[:, (ci % NSLAB) * P:(ci % NSLAB + 1) * P],
                        identity=idb_t[:N_GRAPHS, :N_GRAPHS])
                    hc = fcp.tile([P, N_GRAPHS], BF16, tag="hc")
                    nc.vector.tensor_copy(hc[:], pt16[:])
                    nc.tensor.matmul(
                        out=pfc[:],
                        lhsT=hc[:],
                        rhs=wc[:, j * HID:(j + 1) * HID],
                        start=(ci == 0), stop=(ci == 1023))
            u = sm.tile([N_GRAPHS, HID], F32, tag="u")
            nc.vector.tensor_tensor(out=u[:], in0=pfc[:], in1=bfc1_t[:],
                                    op=mybir.AluOpType.add)
            rlu = sm.tile([N_GRAPHS, HID], F32, tag="rlu")
            nc.scalar.activation(rlu[:], u[:],
                                 mybir.ActivationFunctionType.Relu)
            mnu = sm.tile([N_GRAPHS, HID], F32, tag="mnu")
            nc.vector.scalar_tensor_tensor(
                out=mnu[:], in0=u[:], scalar=0.0, in1=rlu[:],
                op0=mybir.AluOpType.add, op1=mybir.AluOpType.subtract)
            exu = sm.tile([N_GRAPHS, HID], F32, tag="exu")
            nc.scalar.activation(exu[:], mnu[:],
                                 mybir.ActivationFunctionType.Exp)
            fcm = sm.tile([N_GRAPHS, HID], F32, tag="fcm")
            nc.vector.scalar_tensor_tensor(
                out=fcm[:], in0=rlu[:], scalar=-1.0, in1=exu[:],
                op0=mybir.AluOpType.add, op1=mybir.AluOpType.add)
            # transpose [16, 32] -> [32, 16]
            pT = psfc.tile([HID, N_GRAPHS], F32, tag="pT")
            nc.tensor.transpose(out=pT[:], in_=fcm[:],
                                identity=idf_t[:N_GRAPHS, :N_GRAPHS])
            fcmT = sm.tile([HID, N_GRAPHS], F32, tag="fcmT")
            nc.vector.tensor_copy(fcmT[:], pT[:])
            pP = psfc.tile([N_GRAPHS, LATENT], F32, tag="pP")
            nc.tensor.matmul(out=pP[:], lhsT=fcmT[:], rhs=wfc2_t[:],
                             start=True, stop=True)
            part = sm.tile([N_GRAPHS, LATENT], F32, tag="part")
            nc.vector.tensor_copy(part[:], pP[:])
            nc.sync.dma_start(out=arin.ap(), in_=part[:])
            nc.gpsimd.collective_compute(
                "AllReduce", mybir.AluOpType.add, replica_groups=groups,
                ins=[arin.ap().opt()], outs=[arout.ap().opt()])
            res = sm.tile([N_GRAPHS, LATENT], F32, tag="res")
            nc.sync.dma_start(out=res[:], in_=arout.ap())
            fin = sm.tile([N_GRAPHS, LATENT], F32, tag="fin")
            nc.vector.tensor_tensor(out=fin[:], in0=res[:], in1=bfc2_t[:],
                                    op=mybir.AluOpType.add)
            nc.sync.dma_start(out=out.ap(), in_=fin[:])

    nc.compile()
    return nc


def _host_prep(inputs):
    x = np.asarray(inputs["x"], np.float32)
    ei = np.asarray(inputs["edge_index"])
    w = np.asarray(inputs["edge_attr"], np.float32)
    W1 = np.asarray(inputs["W1"], np.float32)
    b1 = np.asarray(inputs["b1"], np.float32)
    W2 = np.asarray(inputs["W2"], np.float32)
    b2 = np.asarray(inputs["b2"], np.float32)
    W3 = np.asarray(inputs["W3"], np.float32)
    b3 = np.asarray(inputs["b3"], np.float32)
    Wfc1 = np.asarray(inputs["Wfc1"], np.float32)
    bfc1 = np.asarray(inputs["bfc1"], np.float32)
    Wfc2 = np.asarray(inputs["Wfc2"], np.float32)
    bfc2 = np.asarray(inputs["bfc2"], np.float32)

    src = ei[0].astype(np.int64)
    dst = ei[1].astype(np.int64)
    E = src.shape[0]

    order = np.argsort(dst, kind="stable")
    d_s = dst[order]
    s_s = src[order]
    w_s = w[order]
    deg = np.bincount(d_s, minlength=N)
    PAD = 8
    while PAD < int(deg.max()):
        PAD *= 2
    starts = np.zeros(N + 1, np.int64)
    np.cumsum(deg, out=starts[1:])
    pos = np.arange(E, dtype=np.int64) - starts[d_s]

    slot_idx = np.full((N, PAD), N, dtype=np.int32)
    slot_w = np.zeros((N, PAD), dtype=np.float32)
    slot_idx[d_s, pos] = s_s.astype(np.int32)
    slot_w[d_s, pos] = w_s

    # [core, chunk, b, p, s] -> [core, chunk, p, s, b]
    si = slot_idx.reshape(NC, NCHUNK, NB, P, PAD).transpose(0, 1, 3, 4, 2)
    idx_arr = np.ascontiguousarray(si.reshape(NC, NCHUNK, P, PAD * NB))
    sw = slot_w.reshape(NC, NCHUNK, NB, P, PAD).transpose(0, 1, 3, 4, 2)
    w_arr = np.ascontiguousarray(
        sw.reshape(NC, NCHUNK, P, PAD * NB)).astype(ml_dtypes.bfloat16)

    z1 = x @ W1
    tab1 = np.zeros((N + 1, HID), ml_dtypes.bfloat16)
    tab1[:N] = z1.astype(ml_dtypes.bfloat16)

    def blockdiag(W):
        out = np.zeros((P, P), np.float32)
        for t in range(NB):
            out[t * HID:(t + 1) * HID, t * HID:(t + 1) * HID] = W
        return out.astype(ml_dtypes.bfloat16)

    wbd = np.stack([blockdiag(W2), blockdiag(W3)])
    bst = np.stack([np.tile(b1, NB), np.tile(b2, NB),
                    np.tile(b3, NB)]).astype(np.float32)
    identf = np.eye(P, dtype=np.float32)

    in_maps = []
    for k in range(NC):
        wfc1s = np.ascontiguousarray(
            Wfc1[:, HID * k:HID * (k + 1)].reshape(1024, P, HID)
            .transpose(1, 0, 2).reshape(P, 1024 * HID)).astype(
                ml_dtypes.bfloat16)
        in_maps.append({
            "tab1": tab1,
            "idx": idx_arr[k],
            "warr": w_arr[k],
            "wbd": wbd,
            "bst": bst,
            "identf": identf,
            "identb": np.eye(P, dtype=ml_dtypes.bfloat16),
            "wfc1s": wfc1s,
            "wfc2s": np.ascontiguousarray(Wfc2[HID * k:HID * (k + 1), :]),
            "bfc1t": np.tile(bfc1[HID * k:HID * (k + 1)], (N_GRAPHS, 1)),
            "bfc2t": np.tile(bfc2, (N_GRAPHS, 1)),
        })
    return PAD, in_maps


def kernel(**inputs):
    PAD, in_maps = _host_prep(inputs)
    if PAD not in _prog_cache:
        _prog_cache[PAD] = _build_program(PAD)
    nc = _prog_cache[PAD]
    res = bass_utils.run_bass_kernel_spmd(nc, in_maps,
                                          core_ids=list(range(NC)))
    return np.asarray(res.results[0]["out"], np.float32)
